# revision 22
# baseline (speedup 1.0000x reference)
"""Trainium2 Bass kernel for dist-biased multi-head attention.

Reference computation (jax):
    qkv = x @ w_qkv; q,k,v = split(qkv); heads of 64
    dots = einsum('bhnd,bhmd->bhnm', q, k) * scale + dist
    attn = softmax(dots, axis=-1)
    out  = einsum('bhnm,bhmd->bhnd', attn, v) -> merge heads -> @ w_out + b_out

Shapes: x [4, 2048, 512], dist [4, 8, 2048, 2048], w_qkv [512, 1536],
w_out [512, 512], b_out [512].

Sharding over 8 cores: core m handles batch m//2, heads 4*(m%2) .. +4.
Each core computes its 4 heads' attention plus the partial out-projection
for its batch; host sums the two partials per batch and adds b_out.

v3 design notes (per-core), informed by NTFF traces:
 - ALL matmuls in bf16: fp32r/fp32/fp16 matmuls are power-throttled to ~50%
   utilization on this hardware; bf16 streams ~2x faster sustained.
 - the NC power governor also clamps the PE when total engine power is high
   (measured: dense real-matmul stream + busy DVE -> 561 ns/MM sustained vs
   265 ns when 1/3 of the stream is near-zero-power identity matmuls and
   DVE is idle). So the dist add uses PE identity matmuls (dist in bf16):
   they are cheap filler in the PE stream and keep DVE cool.
 - scores computed transposed: S^T [keys(part), queries(free)] so attn@v
   contracts keys on the partition dim with no transposes. Softmax skips
   max-subtraction (logits are O(30); exp fits f32/bf16 range) and the
   denominator comes from a ones-column appended to v (row 64 of AV psum).
 - loop h -> kb -> qc so each dist DMA is a [128, 2048] fp16 tile (4KB
   contiguous rows); DMAs round-robin sync/gpsimd queues to engage more
   DMA engines (measured 261 GB/s vs 180 single-queue).
 - q/k projections are pair-packed: stationary [128, 128] = [wq_h | wk_h]
   per contraction chunk; the k half is evacuated from psum partitions
   64:128 to the kT tile at partitions 0:64 (ACT/DVE handle differing
   in/out partition bases fine — only custom DVE uops do not).
 - out-projection pair-stacked: oTp [128 = head-pair, tok] x wo2 [128, 512]
   accumulates both heads of a pair in one matmul (32 instead of 64 MMs).
 - normalization: po psum is evacuated to SBUF immediately (frees the bank
   for the next head); 1/den = exp(-ln(den)) on ACT (Ln/Exp share one
   activation table; reciprocal_approx_fast mishandles partition offsets);
   the normalize multiply runs on gpsimd (all-SBUF operands) to keep DVE
   free for the dist adds.
"""

import numpy as np

N_CORES = 8
B = 4
NTOK = 2048
DIM = 512
HEADS = 8
DH = 64  # head dim
NH = HEADS // 2  # heads per core (4)
NPAIR = NH // 2
INNER = HEADS * DH
SCALE = DH ** -0.5
NKB = NTOK // 128  # key blocks of 128


def _build_nc(variant="v3"):
    import concourse.bacc as bacc
    import concourse.mybir as mybir
    import concourse.tile as tile
    from concourse.bass import ts

    f32 = mybir.dt.float32
    f16 = mybir.dt.float16
    bf16 = mybir.dt.bfloat16
    Exp = mybir.ActivationFunctionType.Exp
    Ln = mybir.ActivationFunctionType.Ln

    # dist tiles with kb % dve_every == dve_every-1 add on DVE instead of the
    # PE identity matmul (0 disables). Balances PE matmul count vs NC power.
    dve_every = 0
    for tok in variant.split("-"):
        if tok.startswith("dve"):
            dve_every = int(tok[3:])

    nc = bacc.Bacc("TRN2", target_bir_lowering=False, debug=False)

    xT_d = nc.dram_tensor("xT", [DIM, NTOK], bf16, kind="ExternalInput").ap()
    # [dim, head, q64|k64]
    wqk_d = nc.dram_tensor("wqk", [DIM, NH, 2 * DH], bf16, kind="ExternalInput").ap()
    wv_d = nc.dram_tensor("wv", [DIM, NH * DH], bf16, kind="ExternalInput").ap()
    distT_d = nc.dram_tensor("distT", [NH, NTOK, NTOK], bf16, kind="ExternalInput").ap()
    # [pair, h0 64d | h1 64d, dim]
    wo_d = nc.dram_tensor("wo", [NPAIR, 2 * DH, DIM], bf16, kind="ExternalInput").ap()
    part_d = nc.dram_tensor("part", [NTOK, DIM], f32, kind="ExternalOutput").ap()

    with tile.TileContext(nc) as tc:
        with (
            tc.tile_pool(name="consts", bufs=1) as consts,
            tc.tile_pool(name="qkv", bufs=1) as qkv,
        ):
            from concourse.masks import make_identity

            ident32 = consts.tile([128, 128], f32)
            make_identity(nc, ident32)
            ident = consts.tile([128, 128], bf16)
            nc.scalar.copy(ident[:], ident32[:])

            wqk_sb = consts.tile([128, DIM // 128, NH, 2 * DH], bf16)
            nc.sync.dma_start(
                wqk_sb[:], wqk_d.rearrange("(c p) h d -> p c h d", p=128)
            )
            wv_sb = consts.tile([128, DIM // 128, NH * DH], bf16)
            nc.sync.dma_start(wv_sb[:], wv_d.rearrange("(c p) n -> p c n", p=128))
            wo_sb = consts.tile([128, NPAIR, DIM], bf16)
            nc.sync.dma_start(wo_sb[:], wo_d.rearrange("t p n -> p t n"))
            # xT chunked per contraction block so projections start early
            xT_r = xT_d.rearrange("(c p) n -> p c n", p=128)
            xT_sb = consts.tile([128, DIM // 128, NTOK], bf16)
            for c in range(DIM // 128):
                eng = nc.sync if c % 2 == 0 else nc.gpsimd
                eng.dma_start(xT_sb[:, c, :], xT_r[:, c, :])

            qT_sb = qkv.tile([DH, NH, NTOK], bf16)
            kT_sb = qkv.tile([DH, NH, NTOK], bf16)
            v_sb = qkv.tile([128, NH, NKB, DH + 1], bf16)
            oTp_sb = qkv.tile([128, NPAIR, NTOK], bf16)
            ones32 = consts.tile([128, NH, NKB, 1], f32)
            nc.gpsimd.memset(ones32[:], 1.0)
            nc.scalar.copy(v_sb[:, :, :, DH : DH + 1], ones32[:])

            # ---- phase 1: projections (bf16), q/k pair-packed ----
            with (
                tc.tile_pool(name="p1qk", bufs=3, space="PSUM") as p1qk,
                tc.tile_pool(name="p1v", bufs=2, space="PSUM") as p1v,
            ):
                for h in range(NH):
                    for half in range(2):
                        ps_qk = p1qk.tile([128, 1024], f32)
                        for c in range(DIM // 128):
                            for j in range(2):
                                nc.tensor.matmul(
                                    ps_qk[:, ts(j, 512)],
                                    wqk_sb[:, c, h, :],
                                    xT_sb[:, c, half * 1024 + 512 * j : half * 1024 + 512 * (j + 1)],
                                    start=(c == 0),
                                    stop=(c == DIM // 128 - 1),
                                )
                        nc.scalar.copy(qT_sb[:, h, ts(half, 1024)], ps_qk[0:DH, :])
                        nc.vector.tensor_copy(
                            kT_sb[:, h, ts(half, 1024)], ps_qk[DH : 2 * DH, :]
                        )
                # v in natural [token, d] layout, all 4 heads at once (N=256)
                for i in range(NKB):
                    ps_v = p1v.tile([128, NH * DH], f32)
                    for c in range(DIM // 128):
                        nc.tensor.matmul(
                            ps_v[:],
                            xT_sb[:, c, ts(i, 128)],
                            wv_sb[:, c, :],
                            start=(c == 0),
                            stop=(c == DIM // 128 - 1),
                        )
                    nc.scalar.copy(
                        v_sb[:, :, i, 0:DH],
                        ps_v.rearrange("p (h d) -> p h d", h=NH),
                    )

            # ---- phase 2: attention ----
            with (
                tc.tile_pool(name="spsum", bufs=2, space="PSUM") as spsum,
                tc.tile_pool(name="opsum", bufs=1, space="PSUM") as opsum,
                tc.tile_pool(name="dist", bufs=6) as distp,
                tc.tile_pool(name="expp", bufs=4) as expp,
                tc.tile_pool(name="otf", bufs=2) as otfp,
                tc.tile_pool(name="smalls", bufs=4) as smalls,
            ):
                for h in range(NH):
                    po = opsum.tile([DH + 1, NTOK], f32)
                    for kb in range(NKB):
                        dt_t = distp.tile([128, NTOK], bf16)
                        eng = nc.sync if kb % 2 == 0 else nc.gpsimd
                        eng.dma_start(dt_t[:], distT_d[h, ts(kb, 128), :])
                        ex = expp.tile([128, NTOK], bf16)
                        dve_add = dve_every > 0 and kb % dve_every == dve_every - 1
                        for qc in range(2):
                            ps = spsum.tile([128, 1024], f32)
                            for j in range(2):
                                nc.tensor.matmul(
                                    ps[:, ts(j, 512)],
                                    kT_sb[:, h, ts(kb, 128)],
                                    qT_sb[:, h, qc * 1024 + 512 * j : qc * 1024 + 512 * (j + 1)],
                                    start=True,
                                    stop=dve_add,
                                )
                            if dve_add:
                                nc.vector.tensor_add(
                                    ps[:], ps[:], dt_t[:, ts(qc, 1024)]
                                )
                            else:
                                for j in range(2):
                                    nc.tensor.matmul(
                                        ps[:, ts(j, 512)],
                                        ident[:],
                                        dt_t[:, qc * 1024 + 512 * j : qc * 1024 + 512 * (j + 1)],
                                        start=False,
                                        stop=True,
                                    )
                            nc.scalar.activation(ex[:, ts(qc, 1024)], ps[:], Exp)
                            for j in range(2):
                                nc.tensor.matmul(
                                    po[:, qc * 1024 + 512 * j : qc * 1024 + 512 * (j + 1)],
                                    v_sb[:, h, kb, :],
                                    ex[:, qc * 1024 + 512 * j : qc * 1024 + 512 * (j + 1)],
                                    start=(kb == 0),
                                    stop=(kb == NKB - 1),
                                )
                    # evacuate + normalize per qc-half so the out-projection
                    # for the first half can overlap the second half's chain.
                    # 1/den via DVE reciprocal_approx_fast on a partition-0
                    # copy of the denominator row (keeps ACT on the Exp table:
                    # Ln would thrash the activation table).
                    pair, sub = h // 2, h % 2
                    for half in range(2):
                        hs = ts(half, 1024)
                        otf = otfp.tile([DH, 1024], f32)
                        nc.vector.tensor_copy(otf[:], po[0:DH, hs])
                        den = smalls.tile([1, 1024], f32)
                        nc.scalar.copy(den[:], po[DH : DH + 1, hs])
                        rcp = smalls.tile([1, 1024], f32)
                        nc.vector.reciprocal_approx_fast(rcp[:], den[:])
                        rb = smalls.tile([DH, 1024], f32)
                        nc.gpsimd.partition_broadcast(rb[:], rcp[:])
                        # heads 2p -> rows 0:64, heads 2p+1 -> rows 64:128
                        # (cross-partition-base write is fine for builtins)
                        nc.vector.tensor_mul(
                            oTp_sb[sub * DH : (sub + 1) * DH, pair, hs],
                            otf[:],
                            rb[:],
                        )

            # ---- phase 3: out-projection (bf16, head pairs) ----
            with (
                tc.tile_pool(name="ppsum", bufs=2, space="PSUM") as ppsum,
                tc.tile_pool(name="outp", bufs=3) as outp,
            ):
                for i in range(NTOK // 128):
                    pp = ppsum.tile([128, DIM], f32)
                    for p in range(NPAIR):
                        nc.tensor.matmul(
                            pp[:],
                            oTp_sb[:, p, ts(i, 128)],
                            wo_sb[:, p, :],
                            start=(p == 0),
                            stop=(p == NPAIR - 1),
                        )
                    ob = outp.tile([128, DIM], f32)
                    nc.vector.tensor_copy(ob[:], pp[:])
                    nc.sync.dma_start(part_d[ts(i, 128), :], ob[:])

    nc.compile()
    return nc


_NC_CACHE = {}


def _get_nc(variant=None):
    if variant is None:
        variant = KERNEL_VARIANT
    if variant not in _NC_CACHE:
        _NC_CACHE[variant] = _build_nc(variant)
    return _NC_CACHE[variant]


def make_in_maps(x, dist, w_qkv, w_out):
    """Host-side sharding: per-core input dicts (dtypes match dram decls)."""
    import ml_dtypes

    f16 = np.float16
    bf16 = ml_dtypes.bfloat16
    x = np.asarray(x, dtype=np.float32)
    dist = np.asarray(dist, dtype=np.float32)
    w_qkv = np.asarray(w_qkv, dtype=np.float32)
    w_out = np.asarray(w_out, dtype=np.float32)
    in_maps = []
    for m in range(N_CORES):
        b = m // 2
        h0 = NH * (m % 2)
        wq = w_qkv[:, h0 * DH : (h0 + NH) * DH] * np.float32(SCALE)
        wk = w_qkv[:, INNER + h0 * DH : INNER + (h0 + NH) * DH]
        wv = w_qkv[:, 2 * INNER + h0 * DH : 2 * INNER + (h0 + NH) * DH]
        # [dim, head, q64|k64]
        wqk = np.concatenate(
            [wq.reshape(DIM, NH, DH), wk.reshape(DIM, NH, DH)], axis=2
        )
        # [pair, 128, dim]
        wo = w_out[h0 * DH : (h0 + NH) * DH, :].reshape(NPAIR, 2 * DH, DIM)
        in_maps.append(
            {
                "xT": np.ascontiguousarray(x[b].T).astype(bf16),
                "wqk": np.ascontiguousarray(wqk).astype(bf16),
                "wv": np.ascontiguousarray(wv).astype(bf16),
                "distT": np.ascontiguousarray(
                    dist[b, h0 : h0 + NH].transpose(0, 2, 1)
                ).astype(bf16),
                "wo": np.ascontiguousarray(wo).astype(bf16),
            }
        )
    return in_maps


def assemble(results, b_out):
    """Sum the two per-batch partials and add bias."""
    out = np.empty((B, NTOK, DIM), dtype=np.float32)
    for b in range(B):
        out[b] = results[2 * b]["part"] + results[2 * b + 1]["part"] + b_out
    return out


KERNEL_VARIANT = "v6-dve4"


def cast_in_maps(nc, in_maps):
    """No-op passthrough kept for test.py compatibility (make_in_maps already
    produces correctly-typed arrays)."""
    return in_maps


def kernel(x, dist, w_qkv, w_out, b_out):
    from concourse.bass_utils import run_bass_kernel_spmd

    nc = _get_nc()
    in_maps = make_in_maps(x, dist, w_qkv, w_out)
    res = run_bass_kernel_spmd(nc, in_maps, core_ids=list(range(N_CORES)))
    return assemble(res.results, np.asarray(b_out, dtype=np.float32))


# revision 23
# speedup vs baseline: 1.1095x; 1.1095x over previous
"""Trainium2 Bass kernel for dist-biased multi-head attention.

Reference computation (jax):
    qkv = x @ w_qkv; q,k,v = split(qkv); heads of 64
    dots = einsum('bhnd,bhmd->bhnm', q, k) * scale + dist
    attn = softmax(dots, axis=-1)
    out  = einsum('bhnm,bhmd->bhnd', attn, v) -> merge heads -> @ w_out + b_out

Shapes: x [4, 2048, 512], dist [4, 8, 2048, 2048], w_qkv [512, 1536],
w_out [512, 512], b_out [512].

Sharding over 8 cores: core m handles batch m//2, heads 4*(m%2) .. +4.
Each core computes its 4 heads' attention plus the partial out-projection
for its batch; host sums the two partials per batch and adds b_out.

v3 design notes (per-core), informed by NTFF traces:
 - ALL matmuls in bf16: fp32r/fp32/fp16 matmuls are power-throttled to ~50%
   utilization on this hardware; bf16 streams ~2x faster sustained.
 - the NC power governor also clamps the PE when total engine power is high
   (measured: dense real-matmul stream + busy DVE -> 561 ns/MM sustained vs
   265 ns when 1/3 of the stream is near-zero-power identity matmuls and
   DVE is idle). So the dist add uses PE identity matmuls (dist in bf16):
   they are cheap filler in the PE stream and keep DVE cool.
 - scores computed transposed: S^T [keys(part), queries(free)] so attn@v
   contracts keys on the partition dim with no transposes. Softmax skips
   max-subtraction (logits are O(30); exp fits f32/bf16 range) and the
   denominator comes from a ones-column appended to v (row 64 of AV psum).
 - loop h -> kb -> qc so each dist DMA is a [128, 2048] fp16 tile (4KB
   contiguous rows); DMAs round-robin sync/gpsimd queues to engage more
   DMA engines (measured 261 GB/s vs 180 single-queue).
 - q/k projections are pair-packed: stationary [128, 128] = [wq_h | wk_h]
   per contraction chunk; the k half is evacuated from psum partitions
   64:128 to the kT tile at partitions 0:64 (ACT/DVE handle differing
   in/out partition bases fine — only custom DVE uops do not).
 - out-projection pair-stacked: oTp [128 = head-pair, tok] x wo2 [128, 512]
   accumulates both heads of a pair in one matmul (32 instead of 64 MMs).
 - normalization: po psum is evacuated to SBUF immediately (frees the bank
   for the next head); 1/den = exp(-ln(den)) on ACT (Ln/Exp share one
   activation table; reciprocal_approx_fast mishandles partition offsets);
   the normalize multiply runs on gpsimd (all-SBUF operands) to keep DVE
   free for the dist adds.
"""

import numpy as np

N_CORES = 8
B = 4
NTOK = 2048
DIM = 512
HEADS = 8
DH = 64  # head dim
NH = HEADS // 2  # heads per core (4)
NPAIR = NH // 2
INNER = HEADS * DH
SCALE = DH ** -0.5
NKB = NTOK // 128  # key blocks of 128


def _build_nc(variant="v3"):
    import concourse.bacc as bacc
    import concourse.mybir as mybir
    import concourse.tile as tile
    from concourse.bass import ts

    f32 = mybir.dt.float32
    f16 = mybir.dt.float16
    bf16 = mybir.dt.bfloat16
    Exp = mybir.ActivationFunctionType.Exp
    Ln = mybir.ActivationFunctionType.Ln

    # dist tiles with kb % dve_every == dve_every-1 add on DVE instead of the
    # PE identity matmul (0 disables). Balances PE matmul count vs NC power.
    dve_every = 0
    for tok in variant.split("-"):
        if tok.startswith("dve"):
            dve_every = int(tok[3:])

    nc = bacc.Bacc("TRN2", target_bir_lowering=False, debug=False)

    xT_d = nc.dram_tensor("xT", [DIM, NTOK], bf16, kind="ExternalInput").ap()
    # [dim, head, q64|k64]
    wqk_d = nc.dram_tensor("wqk", [DIM, NH, 2 * DH], bf16, kind="ExternalInput").ap()
    wv_d = nc.dram_tensor("wv", [DIM, NH * DH], bf16, kind="ExternalInput").ap()
    distT_d = nc.dram_tensor("distT", [NH, NTOK, NTOK], bf16, kind="ExternalInput").ap()
    # [pair, h0 64d | h1 64d, dim]
    wo_d = nc.dram_tensor("wo", [NPAIR, 2 * DH, DIM], bf16, kind="ExternalInput").ap()
    part_d = nc.dram_tensor("part", [NTOK, DIM], f32, kind="ExternalOutput").ap()

    with tile.TileContext(nc) as tc:
        with (
            tc.tile_pool(name="consts", bufs=1) as consts,
            tc.tile_pool(name="qkv", bufs=1) as qkv,
        ):
            from concourse.masks import make_identity

            ident32 = consts.tile([128, 128], f32)
            make_identity(nc, ident32)
            ident = consts.tile([128, 128], bf16)
            nc.scalar.copy(ident[:], ident32[:])

            wqk_sb = consts.tile([128, DIM // 128, NH, 2 * DH], bf16)
            nc.sync.dma_start(
                wqk_sb[:], wqk_d.rearrange("(c p) h d -> p c h d", p=128)
            )
            wv_sb = consts.tile([128, DIM // 128, NH * DH], bf16)
            nc.sync.dma_start(wv_sb[:], wv_d.rearrange("(c p) n -> p c n", p=128))
            wo_sb = consts.tile([128, NPAIR, DIM], bf16)
            nc.sync.dma_start(wo_sb[:], wo_d.rearrange("t p n -> p t n"))
            # xT chunked per contraction block so projections start early
            xT_r = xT_d.rearrange("(c p) n -> p c n", p=128)
            xT_sb = consts.tile([128, DIM // 128, NTOK], bf16)
            for c in range(DIM // 128):
                eng = nc.sync if c % 2 == 0 else nc.gpsimd
                eng.dma_start(xT_sb[:, c, :], xT_r[:, c, :])

            qT_sb = qkv.tile([DH, NH, NTOK], bf16)
            kT_sb = qkv.tile([DH, NH, NTOK], bf16)
            v_sb = qkv.tile([128, NH, NKB, DH + 1], bf16)
            oTp_sb = qkv.tile([128, NPAIR, NTOK], bf16)
            ones32 = consts.tile([128, NH, NKB, 1], f32)
            nc.gpsimd.memset(ones32[:], 1.0)
            nc.scalar.copy(v_sb[:, :, :, DH : DH + 1], ones32[:])

            # ---- phase 1: projections (bf16), q/k pair-packed ----
            with (
                tc.tile_pool(name="p1qk", bufs=3, space="PSUM") as p1qk,
                tc.tile_pool(name="p1v", bufs=2, space="PSUM") as p1v,
            ):
                for h in range(NH):
                    for half in range(2):
                        ps_qk = p1qk.tile([128, 1024], f32)
                        for c in range(DIM // 128):
                            for j in range(2):
                                nc.tensor.matmul(
                                    ps_qk[:, ts(j, 512)],
                                    wqk_sb[:, c, h, :],
                                    xT_sb[:, c, half * 1024 + 512 * j : half * 1024 + 512 * (j + 1)],
                                    start=(c == 0),
                                    stop=(c == DIM // 128 - 1),
                                )
                        nc.scalar.copy(qT_sb[:, h, ts(half, 1024)], ps_qk[0:DH, :])
                        nc.vector.tensor_copy(
                            kT_sb[:, h, ts(half, 1024)], ps_qk[DH : 2 * DH, :]
                        )
                # v in natural [token, d] layout, all 4 heads at once (N=256)
                for i in range(NKB):
                    ps_v = p1v.tile([128, NH * DH], f32)
                    for c in range(DIM // 128):
                        nc.tensor.matmul(
                            ps_v[:],
                            xT_sb[:, c, ts(i, 128)],
                            wv_sb[:, c, :],
                            start=(c == 0),
                            stop=(c == DIM // 128 - 1),
                        )
                    nc.scalar.copy(
                        v_sb[:, :, i, 0:DH],
                        ps_v.rearrange("p (h d) -> p h d", h=NH),
                    )

            # ---- phase 2: attention ----
            with (
                tc.tile_pool(name="spsum", bufs=2, space="PSUM") as spsum,
                tc.tile_pool(name="opsum", bufs=1, space="PSUM") as opsum,
                tc.tile_pool(name="dist", bufs=6) as distp,
                tc.tile_pool(name="expp", bufs=4) as expp,
                tc.tile_pool(name="otf", bufs=2) as otfp,
                tc.tile_pool(name="smalls", bufs=4) as smalls,
            ):
                for h in range(NH):
                    po = opsum.tile([DH + 1, NTOK], f32)
                    for kb in range(NKB):
                        dt_t = distp.tile([128, NTOK], bf16)
                        eng = nc.sync if kb % 2 == 0 else nc.gpsimd
                        eng.dma_start(dt_t[:], distT_d[h, ts(kb, 128), :])
                        ex = expp.tile([128, NTOK], bf16)
                        dve_add = dve_every > 0 and kb % dve_every == dve_every - 1
                        for qc in range(2):
                            ps = spsum.tile([128, 1024], f32)
                            for j in range(2):
                                nc.tensor.matmul(
                                    ps[:, ts(j, 512)],
                                    kT_sb[:, h, ts(kb, 128)],
                                    qT_sb[:, h, qc * 1024 + 512 * j : qc * 1024 + 512 * (j + 1)],
                                    start=True,
                                    stop=dve_add,
                                )
                            if dve_add:
                                nc.vector.tensor_add(
                                    ps[:], ps[:], dt_t[:, ts(qc, 1024)]
                                )
                            else:
                                for j in range(2):
                                    nc.tensor.matmul(
                                        ps[:, ts(j, 512)],
                                        ident[:],
                                        dt_t[:, qc * 1024 + 512 * j : qc * 1024 + 512 * (j + 1)],
                                        start=False,
                                        stop=True,
                                    )
                            nc.scalar.activation(ex[:, ts(qc, 1024)], ps[:], Exp)
                            for j in range(2):
                                nc.tensor.matmul(
                                    po[:, qc * 1024 + 512 * j : qc * 1024 + 512 * (j + 1)],
                                    v_sb[:, h, kb, :],
                                    ex[:, qc * 1024 + 512 * j : qc * 1024 + 512 * (j + 1)],
                                    start=(kb == 0),
                                    stop=(kb == NKB - 1),
                                )
                    # evacuate + normalize per qc-half so the out-projection
                    # for the first half can overlap the second half's chain.
                    # 1/den via DVE reciprocal_approx_fast on a partition-0
                    # copy of the denominator row (keeps ACT on the Exp table:
                    # Ln would thrash the activation table).
                    pair, sub = h // 2, h % 2
                    for half in range(2):
                        hs = ts(half, 1024)
                        otf = otfp.tile([DH, 1024], f32)
                        nc.vector.tensor_copy(otf[:], po[0:DH, hs])
                        den = smalls.tile([1, 1024], f32)
                        nc.scalar.copy(den[:], po[DH : DH + 1, hs])
                        rcp = smalls.tile([1, 1024], f32)
                        nc.vector.reciprocal_approx_fast(rcp[:], den[:])
                        rb = smalls.tile([DH, 1024], f32)
                        nc.gpsimd.partition_broadcast(rb[:], rcp[:])
                        # heads 2p -> rows 0:64, heads 2p+1 -> rows 64:128
                        # (cross-partition-base write is fine for builtins)
                        nc.vector.tensor_mul(
                            oTp_sb[sub * DH : (sub + 1) * DH, pair, hs],
                            otf[:],
                            rb[:],
                        )

            # ---- phase 3: out-projection (bf16, head pairs) ----
            with (
                tc.tile_pool(name="ppsum", bufs=2, space="PSUM") as ppsum,
                tc.tile_pool(name="outp", bufs=3) as outp,
            ):
                for i in range(NTOK // 128):
                    pp = ppsum.tile([128, DIM], f32)
                    for p in range(NPAIR):
                        nc.tensor.matmul(
                            pp[:],
                            oTp_sb[:, p, ts(i, 128)],
                            wo_sb[:, p, :],
                            start=(p == 0),
                            stop=(p == NPAIR - 1),
                        )
                    ob = outp.tile([128, DIM], f32)
                    nc.vector.tensor_copy(ob[:], pp[:])
                    nc.sync.dma_start(part_d[ts(i, 128), :], ob[:])

    nc.compile()
    return nc


_NC_CACHE = {}


def _get_nc(variant=None):
    if variant is None:
        variant = KERNEL_VARIANT
    if variant not in _NC_CACHE:
        _NC_CACHE[variant] = _build_nc(variant)
    return _NC_CACHE[variant]


def make_in_maps(x, dist, w_qkv, w_out):
    """Host-side sharding: per-core input dicts (dtypes match dram decls)."""
    import ml_dtypes

    f16 = np.float16
    bf16 = ml_dtypes.bfloat16
    x = np.asarray(x, dtype=np.float32)
    dist = np.asarray(dist, dtype=np.float32)
    w_qkv = np.asarray(w_qkv, dtype=np.float32)
    w_out = np.asarray(w_out, dtype=np.float32)
    in_maps = []
    for m in range(N_CORES):
        b = m // 2
        h0 = NH * (m % 2)
        wq = w_qkv[:, h0 * DH : (h0 + NH) * DH] * np.float32(SCALE)
        wk = w_qkv[:, INNER + h0 * DH : INNER + (h0 + NH) * DH]
        wv = w_qkv[:, 2 * INNER + h0 * DH : 2 * INNER + (h0 + NH) * DH]
        # [dim, head, q64|k64]
        wqk = np.concatenate(
            [wq.reshape(DIM, NH, DH), wk.reshape(DIM, NH, DH)], axis=2
        )
        # [pair, 128, dim]
        wo = w_out[h0 * DH : (h0 + NH) * DH, :].reshape(NPAIR, 2 * DH, DIM)
        in_maps.append(
            {
                "xT": np.ascontiguousarray(x[b].T).astype(bf16),
                "wqk": np.ascontiguousarray(wqk).astype(bf16),
                "wv": np.ascontiguousarray(wv).astype(bf16),
                "distT": np.ascontiguousarray(
                    dist[b, h0 : h0 + NH].transpose(0, 2, 1)
                ).astype(bf16),
                "wo": np.ascontiguousarray(wo).astype(bf16),
            }
        )
    return in_maps


def assemble(results, b_out):
    """Sum the two per-batch partials and add bias."""
    out = np.empty((B, NTOK, DIM), dtype=np.float32)
    for b in range(B):
        out[b] = results[2 * b]["part"] + results[2 * b + 1]["part"] + b_out
    return out


# Hybrid DVE dist-adds measured SLOWER (dve4: 385us vs 294.8us) — any
# meaningful DVE load re-trips the NC power governor's PE clamp. All adds
# stay on the PE as identity matmuls.
KERNEL_VARIANT = "v5"


def cast_in_maps(nc, in_maps):
    """No-op passthrough kept for test.py compatibility (make_in_maps already
    produces correctly-typed arrays)."""
    return in_maps


def kernel(x, dist, w_qkv, w_out, b_out):
    from concourse.bass_utils import run_bass_kernel_spmd

    nc = _get_nc()
    in_maps = make_in_maps(x, dist, w_qkv, w_out)
    res = run_bass_kernel_spmd(nc, in_maps, core_ids=list(range(N_CORES)))
    return assemble(res.results, np.asarray(b_out, dtype=np.float32))


# revision 27
# speedup vs baseline: 1.2064x; 1.0873x over previous
"""Trainium2 Bass kernel for dist-biased multi-head attention.

Reference computation (jax):
    qkv = x @ w_qkv; q,k,v = split(qkv); heads of 64
    dots = einsum('bhnd,bhmd->bhnm', q, k) * scale + dist
    attn = softmax(dots, axis=-1)
    out  = einsum('bhnm,bhmd->bhnd', attn, v) -> merge heads -> @ w_out + b_out

Shapes: x [4, 2048, 512], dist [4, 8, 2048, 2048], w_qkv [512, 1536],
w_out [512, 512], b_out [512].

Sharding over 8 cores: core m handles batch m//2, heads 4*(m%2) .. +4.
Each core computes its 4 heads' attention plus the partial out-projection
for its batch; host sums the two partials per batch and adds b_out.

v3 design notes (per-core), informed by NTFF traces:
 - ALL matmuls in bf16: fp32r/fp32/fp16 matmuls are power-throttled to ~50%
   utilization on this hardware; bf16 streams ~2x faster sustained.
 - the NC power governor also clamps the PE when total engine power is high
   (measured: dense real-matmul stream + busy DVE -> 561 ns/MM sustained vs
   265 ns when 1/3 of the stream is near-zero-power identity matmuls and
   DVE is idle). So the dist add uses PE identity matmuls (dist in bf16):
   they are cheap filler in the PE stream and keep DVE cool.
 - scores computed transposed: S^T [keys(part), queries(free)] so attn@v
   contracts keys on the partition dim with no transposes. Softmax skips
   max-subtraction (logits are O(30); exp fits f32/bf16 range) and the
   denominator comes from a ones-column appended to v (row 64 of AV psum).
 - loop h -> kb -> qc so each dist DMA is a [128, 2048] fp16 tile (4KB
   contiguous rows); DMAs round-robin sync/gpsimd queues to engage more
   DMA engines (measured 261 GB/s vs 180 single-queue).
 - q/k projections are pair-packed: stationary [128, 128] = [wq_h | wk_h]
   per contraction chunk; the k half is evacuated from psum partitions
   64:128 to the kT tile at partitions 0:64 (ACT/DVE handle differing
   in/out partition bases fine — only custom DVE uops do not).
 - out-projection pair-stacked: oTp [128 = head-pair, tok] x wo2 [128, 512]
   accumulates both heads of a pair in one matmul (32 instead of 64 MMs).
 - normalization: po psum is evacuated to SBUF immediately (frees the bank
   for the next head); 1/den = exp(-ln(den)) on ACT (Ln/Exp share one
   activation table; reciprocal_approx_fast mishandles partition offsets);
   the normalize multiply runs on gpsimd (all-SBUF operands) to keep DVE
   free for the dist adds.
"""

import numpy as np

N_CORES = 8
B = 4
NTOK = 2048
DIM = 512
HEADS = 8
DH = 64  # head dim
NH = HEADS // 2  # heads per core (4)
NPAIR = NH // 2
INNER = HEADS * DH
SCALE = DH ** -0.5
NKB = NTOK // 128  # key blocks of 128


def _build_nc(variant="v3"):
    import concourse.bacc as bacc
    import concourse.mybir as mybir
    import concourse.tile as tile
    from concourse.bass import ts

    f32 = mybir.dt.float32
    f16 = mybir.dt.float16
    bf16 = mybir.dt.bfloat16
    Exp = mybir.ActivationFunctionType.Exp
    Ln = mybir.ActivationFunctionType.Ln

    # dist tiles with kb % dve_every == dve_every-1 add on DVE instead of the
    # PE identity matmul (0 disables). Balances PE matmul count vs NC power.
    dve_every = 0
    for tok in variant.split("-"):
        if tok.startswith("dve"):
            dve_every = int(tok[3:])

    nc = bacc.Bacc("TRN2", target_bir_lowering=False, debug=False)

    xT_d = nc.dram_tensor("xT", [DIM, NTOK], bf16, kind="ExternalInput").ap()
    # [dim, head, q64|k64]
    wqk_d = nc.dram_tensor("wqk", [DIM, NH, 2 * DH], bf16, kind="ExternalInput").ap()
    wv_d = nc.dram_tensor("wv", [DIM, NH * DH], bf16, kind="ExternalInput").ap()
    distT_d = nc.dram_tensor("distT", [NH, NTOK, NTOK], bf16, kind="ExternalInput").ap()
    # [pair, h0 64d | h1 64d, dim]
    wo_d = nc.dram_tensor("wo", [NPAIR, 2 * DH, DIM], bf16, kind="ExternalInput").ap()
    part_d = nc.dram_tensor("part", [NTOK, DIM], f32, kind="ExternalOutput").ap()

    with tile.TileContext(nc) as tc:
        with (
            tc.tile_pool(name="consts", bufs=1) as consts,
            tc.tile_pool(name="qkv", bufs=1) as qkv,
        ):
            from concourse.masks import make_identity

            ident32 = consts.tile([128, 128], f32)
            make_identity(nc, ident32)
            ident = consts.tile([128, 128], bf16)
            nc.scalar.copy(ident[:], ident32[:])

            wqk_sb = consts.tile([128, DIM // 128, NH, 2 * DH], bf16)
            nc.sync.dma_start(
                wqk_sb[:], wqk_d.rearrange("(c p) h d -> p c h d", p=128)
            )
            wv_sb = consts.tile([128, DIM // 128, NH * DH], bf16)
            nc.sync.dma_start(wv_sb[:], wv_d.rearrange("(c p) n -> p c n", p=128))
            wo_sb = consts.tile([128, NPAIR, DIM], bf16)
            nc.sync.dma_start(wo_sb[:], wo_d.rearrange("t p n -> p t n"))
            # xT chunked per contraction block so projections start early
            xT_r = xT_d.rearrange("(c p) n -> p c n", p=128)
            xT_sb = consts.tile([128, DIM // 128, NTOK], bf16)
            for c in range(DIM // 128):
                eng = nc.sync if c % 2 == 0 else nc.gpsimd
                eng.dma_start(xT_sb[:, c, :], xT_r[:, c, :])

            qT_sb = qkv.tile([DH, NH, NTOK], bf16)
            kT_sb = qkv.tile([DH, NH, NTOK], bf16)
            v_sb = qkv.tile([128, NH, NKB, DH + 1], bf16)
            oTp_sb = qkv.tile([128, NPAIR, NTOK], bf16)
            # pair-0 out-projection partials, computed mid-attention
            ob_acc = qkv.tile([128, NTOK // 128, DIM], f32)
            ones32 = consts.tile([128, NH, NKB, 1], f32)
            nc.gpsimd.memset(ones32[:], 1.0)
            nc.scalar.copy(v_sb[:, :, :, DH : DH + 1], ones32[:])

            # ---- phase 1: projections (bf16), q/k pair-packed ----
            with (
                tc.tile_pool(name="p1qk", bufs=3, space="PSUM") as p1qk,
                tc.tile_pool(name="p1v", bufs=2, space="PSUM") as p1v,
            ):
                for h in range(NH):
                    for half in range(2):
                        ps_qk = p1qk.tile([128, 1024], f32)
                        for c in range(DIM // 128):
                            for j in range(2):
                                nc.tensor.matmul(
                                    ps_qk[:, ts(j, 512)],
                                    wqk_sb[:, c, h, :],
                                    xT_sb[:, c, half * 1024 + 512 * j : half * 1024 + 512 * (j + 1)],
                                    start=(c == 0),
                                    stop=(c == DIM // 128 - 1),
                                )
                        nc.scalar.copy(qT_sb[:, h, ts(half, 1024)], ps_qk[0:DH, :])
                        nc.vector.tensor_copy(
                            kT_sb[:, h, ts(half, 1024)], ps_qk[DH : 2 * DH, :]
                        )
                # v in natural [token, d] layout, all 4 heads at once (N=256)
                for i in range(NKB):
                    ps_v = p1v.tile([128, NH * DH], f32)
                    for c in range(DIM // 128):
                        nc.tensor.matmul(
                            ps_v[:],
                            xT_sb[:, c, ts(i, 128)],
                            wv_sb[:, c, :],
                            start=(c == 0),
                            stop=(c == DIM // 128 - 1),
                        )
                    nc.scalar.copy(
                        v_sb[:, :, i, 0:DH],
                        ps_v.rearrange("p (h d) -> p h d", h=NH),
                    )

            # ---- phase 2: attention ----
            with (
                tc.tile_pool(name="spsum", bufs=2, space="PSUM") as spsum,
                tc.tile_pool(name="opsum", bufs=1, space="PSUM") as opsum,
                tc.tile_pool(name="dist", bufs=6) as distp,
                tc.tile_pool(name="expp", bufs=4) as expp,
                tc.tile_pool(name="otf", bufs=2) as otfp,
                tc.tile_pool(name="smalls", bufs=4) as smalls,
            ):
                for h in range(NH):
                    po = opsum.tile([DH + 1, NTOK], f32)
                    for kb in range(NKB):
                        dt_t = distp.tile([128, NTOK], bf16)
                        eng = nc.sync if kb % 2 == 0 else nc.gpsimd
                        eng.dma_start(dt_t[:], distT_d[h, ts(kb, 128), :])
                        ex = expp.tile([128, NTOK], bf16)
                        dve_add = dve_every > 0 and kb % dve_every == dve_every - 1
                        for qc in range(2):
                            ps = spsum.tile([128, 1024], f32)
                            for j in range(2):
                                nc.tensor.matmul(
                                    ps[:, ts(j, 512)],
                                    kT_sb[:, h, ts(kb, 128)],
                                    qT_sb[:, h, qc * 1024 + 512 * j : qc * 1024 + 512 * (j + 1)],
                                    start=True,
                                    stop=dve_add,
                                )
                            if dve_add:
                                nc.vector.tensor_add(
                                    ps[:], ps[:], dt_t[:, ts(qc, 1024)]
                                )
                            else:
                                for j in range(2):
                                    nc.tensor.matmul(
                                        ps[:, ts(j, 512)],
                                        ident[:],
                                        dt_t[:, qc * 1024 + 512 * j : qc * 1024 + 512 * (j + 1)],
                                        start=False,
                                        stop=True,
                                    )
                            nc.scalar.activation(ex[:, ts(qc, 1024)], ps[:], Exp)
                            for j in range(2):
                                nc.tensor.matmul(
                                    po[:, qc * 1024 + 512 * j : qc * 1024 + 512 * (j + 1)],
                                    v_sb[:, h, kb, :],
                                    ex[:, qc * 1024 + 512 * j : qc * 1024 + 512 * (j + 1)],
                                    start=(kb == 0),
                                    stop=(kb == NKB - 1),
                                )
                    # evacuate + normalize per qc-half so the out-projection
                    # for the first half can overlap the second half's chain.
                    # 1/den via DVE reciprocal_approx_fast on a partition-0
                    # copy of the denominator row (keeps ACT on the Exp table:
                    # Ln would thrash the activation table).
                    pair, sub = h // 2, h % 2
                    for half in range(2):
                        hs = ts(half, 1024)
                        otf = otfp.tile([DH, 1024], f32)
                        nc.vector.tensor_copy(otf[:], po[0:DH, hs])
                        den = smalls.tile([1, 1024], f32)
                        nc.scalar.copy(den[:], po[DH : DH + 1, hs])
                        rcp = smalls.tile([1, 1024], f32)
                        nc.vector.reciprocal_approx_fast(rcp[:], den[:])
                        rb = smalls.tile([DH, 1024], f32)
                        nc.gpsimd.partition_broadcast(rb[:], rcp[:])
                        # heads 2p -> rows 0:64, heads 2p+1 -> rows 64:128
                        # (cross-partition-base write is fine for builtins)
                        nc.vector.tensor_mul(
                            oTp_sb[sub * DH : (sub + 1) * DH, pair, hs],
                            otf[:],
                            rb[:],
                        )
                    if h == 1:
                        # pair 0 complete: run its out-projection now, borrowing
                        # the rotating scores psum buffers, so only pair 1's
                        # half remains in the tail after the last head.
                        for i in range(NTOK // 128):
                            ppw = spsum.tile([128, 1024], f32, tag="ps", name="ppw")
                            nc.tensor.matmul(
                                ppw[:, 0:DIM],
                                oTp_sb[:, 0, ts(i, 128)],
                                wo_sb[:, 0, :],
                                start=True,
                                stop=True,
                            )
                            nc.scalar.copy(ob_acc[:, i, :], ppw[:, 0:DIM])

            # ---- phase 3: out-projection (bf16, head pairs) ----
            with (
                tc.tile_pool(name="ppsum", bufs=2, space="PSUM") as ppsum,
                tc.tile_pool(name="outp", bufs=3) as outp,
            ):
                for i in range(NTOK // 128):
                    pp = ppsum.tile([128, DIM], f32)
                    nc.tensor.matmul(
                        pp[:],
                        oTp_sb[:, 1, ts(i, 128)],
                        wo_sb[:, 1, :],
                        start=True,
                        stop=True,
                    )
                    ob = outp.tile([128, DIM], f32)
                    nc.vector.tensor_add(ob[:], ob_acc[:, i, :], pp[:])
                    nc.sync.dma_start(part_d[ts(i, 128), :], ob[:])

    nc.compile()
    return nc


_NC_CACHE = {}


def _get_nc(variant=None):
    if variant is None:
        variant = KERNEL_VARIANT
    if variant not in _NC_CACHE:
        _NC_CACHE[variant] = _build_nc(variant)
    return _NC_CACHE[variant]


def make_in_maps(x, dist, w_qkv, w_out):
    """Host-side sharding: per-core input dicts (dtypes match dram decls)."""
    import ml_dtypes

    f16 = np.float16
    bf16 = ml_dtypes.bfloat16
    x = np.asarray(x, dtype=np.float32)
    dist = np.asarray(dist, dtype=np.float32)
    w_qkv = np.asarray(w_qkv, dtype=np.float32)
    w_out = np.asarray(w_out, dtype=np.float32)
    in_maps = []
    for m in range(N_CORES):
        b = m // 2
        h0 = NH * (m % 2)
        wq = w_qkv[:, h0 * DH : (h0 + NH) * DH] * np.float32(SCALE)
        wk = w_qkv[:, INNER + h0 * DH : INNER + (h0 + NH) * DH]
        wv = w_qkv[:, 2 * INNER + h0 * DH : 2 * INNER + (h0 + NH) * DH]
        # [dim, head, q64|k64]
        wqk = np.concatenate(
            [wq.reshape(DIM, NH, DH), wk.reshape(DIM, NH, DH)], axis=2
        )
        # [pair, 128, dim]
        wo = w_out[h0 * DH : (h0 + NH) * DH, :].reshape(NPAIR, 2 * DH, DIM)
        in_maps.append(
            {
                "xT": np.ascontiguousarray(x[b].T).astype(bf16),
                "wqk": np.ascontiguousarray(wqk).astype(bf16),
                "wv": np.ascontiguousarray(wv).astype(bf16),
                "distT": np.ascontiguousarray(
                    dist[b, h0 : h0 + NH].transpose(0, 2, 1)
                ).astype(bf16),
                "wo": np.ascontiguousarray(wo).astype(bf16),
            }
        )
    return in_maps


def assemble(results, b_out):
    """Sum the two per-batch partials and add bias."""
    out = np.empty((B, NTOK, DIM), dtype=np.float32)
    for b in range(B):
        out[b] = results[2 * b]["part"] + results[2 * b + 1]["part"] + b_out
    return out


# Hybrid DVE dist-adds measured SLOWER (dve4: 385us vs 294.8us) — any
# meaningful DVE load re-trips the NC power governor's PE clamp. All adds
# stay on the PE as identity matmuls.
KERNEL_VARIANT = "v5"


def cast_in_maps(nc, in_maps):
    """No-op passthrough kept for test.py compatibility (make_in_maps already
    produces correctly-typed arrays)."""
    return in_maps


def kernel(x, dist, w_qkv, w_out, b_out):
    from concourse.bass_utils import run_bass_kernel_spmd

    nc = _get_nc()
    in_maps = make_in_maps(x, dist, w_qkv, w_out)
    res = run_bass_kernel_spmd(nc, in_maps, core_ids=list(range(N_CORES)))
    return assemble(res.results, np.asarray(b_out, dtype=np.float32))
